# revision 13
# baseline (speedup 1.0000x reference)
"""Trainium2 Bass kernel for batched single-head attention with projections.

Reference computation (per batch b):
    Q = q @ Wq + bq ; K = k @ Wk + bk ; V = v @ Wv + bv        (512 -> 64)
    out = softmax(Q K^T / 8) V                                  (S = 4096)

Sharding: 8 cores = 4 batches x 2 kv-sequence halves. Each core gets
its full q (transposed, bf16) plus half of k,v for its batch (transposed,
bf16). Cores emit unnormalized numerator + denominator; host combines.

Device-side layout (transposed space):
  Q.T [128, 4096] = (Wq|Wq).T @ qT (+bq)   rows 64..127 duplicate 0..63
  K.T [128, 2048] = (Wk|Wk).T @ kT         (bk dropped: softmax-invariant)
  V'  [2048, 65]  = vT.T @ Wv_aug + bias ; col 64 == 1.0 (denominator col)
  per kv-tile T (128 kv rows x 512 q): scores.T -> PSUM, exp -> bf16 SBUF,
  V'.T @ P.T accumulated into [65, 512] per q-block.

Perf structure (v3):
  - every matmul with contraction 64 is row-tiled (tile_position) so pairs
    run concurrently: scores tiles pair by parity; each AV matmul splits
    into kv-halves accumulating into two PSUM banks (summed by one DVE
    add in the epilogue). Row-disjoint LDWEIGHTS pull ahead of in-flight
    matmuls, hiding the weight-load cost that serialized v2.
  - scores PSUM = two 2-bank slots; one ACTIVATE(exp) per 2-tile group
    (N=1024) ping-ponging the slots -> ScalarE (the floor engine, ~65us
    of exp) stays saturated.
  - PE warm-up matmuls on memset data flip the HAM clock gate to 2.4 GHz
    before the first input DMA lands.
  - input DMAs fan out across rings at t=0 (K+Q-tail on sync, Q-block0 on
    scalar, weights+V+output stores on gpsimd); ScalarE does exps only.
"""

import numpy as np
import ml_dtypes

import concourse.bass as bass
import concourse.tile as tile
from concourse import mybir
from concourse.bass_utils import run_bass_kernel_spmd

BF16 = mybir.dt.bfloat16
F32 = mybir.dt.float32

B, S, D, E = 4, 4096, 512, 64
H = S                 # q rows per core (full sequence)
KS = S // 2           # kv rows per core (half sequence)
E1 = E + 1            # V' width (ones column appended)
NCH = D // 128        # contraction chunks (4)
NKV = KS // 128       # kv tiles per core (16)
QBLK = 512            # q columns per block
NBLK = H // QBLK      # 8
NT = NBLK * NKV       # global tile count (128)
GRP = 2               # kv-tiles per exp group / psum slot
NGRP = NT // GRP      # 64
N_CORES = 8


def _build_bass(split_waits: bool = True) -> bass.Bass:
    nc = bass.Bass()
    qT = nc.declare_dram_parameter("qT", [D, H], BF16, isOutput=False)
    kT = nc.declare_dram_parameter("kT", [D, KS], BF16, isOutput=False)
    vT = nc.declare_dram_parameter("vT", [D, KS], BF16, isOutput=False)
    # weights pre-swizzled on host to [128, chunk*width] (partition-major);
    # wq/wk have their 64 columns duplicated -> 128-wide stationary
    wq = nc.declare_dram_parameter("wq", [128, NCH * 2 * E], BF16, isOutput=False)
    wk = nc.declare_dram_parameter("wk", [128, NCH * 2 * E], BF16, isOutput=False)
    wv = nc.declare_dram_parameter("wv", [128, NCH * E1], BF16, isOutput=False)
    bq = nc.declare_dram_parameter("bqb", [128, QBLK], BF16, isOutput=False)
    bvb = nc.declare_dram_parameter("bvb", [128, E1], F32, isOutput=False)
    out = nc.declare_dram_parameter("out", [E1, H], F32, isOutput=True)

    with tile.TileContext(nc) as tc:
        _body(nc, tc, qT, kT, vT, wq, wk, wv, bq, bvb, out)
    if split_waits:
        _split_multi_waits(nc)
    return nc


_NO_SPLIT_OPCODES = {"Drain", "EventSemaphore", "NoOp", "Call", "ISA",
                     "UnconditionalBranch"}


def _split_multi_waits(nc):
    """walrus (this toolchain) encodes at most ONE sem wait per TPB
    instruction (single NEURON_ISA_TPB_EVENTS slot) and refuses to compile
    instructions carrying more. Tile emits multi-wait sync_info freely, so
    split: keep the first wait on the instruction, hoist the rest onto
    standalone EventSemaphore waits just before it on the same engine."""
    n = 0
    for blk in nc.m.functions[0].blocks:
        new_insts = []
        for inst in blk.instructions:
            si = inst.sync_info
            if (si is not None and si.on_wait and len(si.on_wait) > 1
                    and inst.concise_opcode not in _NO_SPLIT_OPCODES):
                waits = list(si.on_wait)
                for w in waits[:-1]:
                    n += 1
                    es = mybir.InstEventSemaphore(
                        name=f"WSPLIT-{n}", ins=[], outs=[])
                    es.engine = inst.engine
                    es.sync_info = mybir.SyncInfo(on_wait=[w], on_update=[])
                    new_insts.append(es)
                inst.sync_info = mybir.SyncInfo(
                    on_wait=[waits[-1]], on_update=list(si.on_update))
            new_insts.append(inst)
        blk.instructions = new_insts
    return nc


def _body(nc, tc, qT, kT, vT, wq, wk, wv, bq, bvb, out):
    with (
        tc.tile_pool(name="consts", bufs=1) as cst,
        tc.tile_pool(name="raw", bufs=1) as raw,
        tc.tile_pool(name="proj", bufs=1) as proj,
        tc.tile_pool(name="pt", bufs=4) as ptp,
        tc.tile_pool(name="ob", bufs=2) as obp,
        tc.tile_pool(name="sc", bufs=2, space="PSUM") as scp,
        tc.tile_pool(name="accu", bufs=1, space="PSUM") as accup,
        tc.tile_pool(name="accl", bufs=1, space="PSUM") as acclp,
        tc.tile_pool(name="pp", bufs=2, space="PSUM") as ppp,
    ):
        # --- input DMAs, fanned out across rings so descriptor generation
        # (~0.8us per dma_start per ring) and the per-row FIFO transfer
        # queues drain in need-order.
        def load3d(eng, name, src, c0, c1):
            w = c1 - c0
            t = raw.tile([128, NCH, w], BF16, tag=name, name=name)
            eng.dma_start(
                out=t,
                in_=src[:, c0:c1].rearrange("(c p) w -> p c w", p=128))
            return t

        # gpsimd (SWDGE): small weight tensors, then V
        wk_sb = cst.tile([128, NCH * 2 * E], BF16, tag="wk")
        nc.gpsimd.dma_start(out=wk_sb, in_=wk[:, :])
        wq_sb = cst.tile([128, NCH * 2 * E], BF16, tag="wq")
        nc.gpsimd.dma_start(out=wq_sb, in_=wq[:, :])
        bq_sb = cst.tile([128, QBLK], BF16, tag="bq")
        nc.gpsimd.dma_start(out=bq_sb, in_=bq[:, :])
        wv_sb = cst.tile([128, NCH * E1], BF16, tag="wv")
        nc.gpsimd.dma_start(out=wv_sb, in_=wv[:, :])
        bvb_sb = cst.tile([128, E1], F32, tag="bvb")
        nc.gpsimd.dma_start(out=bvb_sb, in_=bvb[:, :])
        vq0 = load3d(nc.gpsimd, "vq0", vT, 0, 1024)
        vq1 = load3d(nc.gpsimd, "vq1", vT, 1024, 2048)

        # sync ring: K in need-order (first 256 kv cols alone so the first
        # scores group starts early), then the q tail
        kqa = load3d(nc.sync, "kqa", kT, 0, 256)
        kqb = load3d(nc.sync, "kqb", kT, 256, 512)
        kq1 = load3d(nc.sync, "kq1", kT, 512, 1024)
        kq2 = load3d(nc.sync, "kq2", kT, 1024, 1536)
        kq3 = load3d(nc.sync, "kq3", kT, 1536, 2048)
        qt_r1 = load3d(nc.sync, "qt_r1", qT, 512, 2048)
        qt_r2 = load3d(nc.sync, "qt_r2", qT, 2048, H)

        # scalar ring: q block 0 only; ScalarE then does exps exclusively
        qt_b0 = load3d(nc.scalar, "qt_b0", qT, 0, 512)

        # PE warm-up: ~3.5us of matmuls on memset garbage flips the HAM
        # clock gate (1.2 -> 2.4 GHz) while the input DMAs are in flight.
        warm = cst.tile([128, 256], BF16, tag="warm")
        nc.vector.memset(warm, 0.0)
        wacc = ppp.tile([128, QBLK], F32, tag="pp", name="wacc")
        for i in range(12):
            nc.tensor.matmul(wacc[:, 0:256], warm[:, 0:128], warm[:, :],
                             start=(i == 0), stop=(i == 11))

        # preload the exp table set (first real exp otherwise eats the
        # ~2.7us ACT_TABLE_LOAD mid-pipeline); reads memset data
        scr = cst.tile([1, 8], F32, tag="scr")
        nc.scalar.activation(scr[:, :], warm[0:1, 0:8],
                             mybir.ActivationFunctionType.Exp)

        def qt_slice(c, blk):
            if blk == 0:
                return qt_b0[:, c, :]
            if blk < 4:
                return qt_r1[:, c, (blk - 1) * 512:blk * 512]
            return qt_r2[:, c, (blk - 4) * 512:(blk - 3) * 512]

        def vt_slice(c, t):         # 128-col V tile
            return (vq0 if t < 8 else vq1)[:, c, (t % 8) * 128:(t % 8 + 1) * 128]

        # projected tensors; Q.T/K.T rows 64..127 duplicate rows 0..63 (via
        # column-duplicated weights) so scores matmuls can row-pack.
        QT2 = proj.tile([128, H], BF16, tag="QT2")
        KT2 = proj.tile([128, KS], BF16, tag="KT2")
        Vp = proj.tile([128, NKV, E1], BF16, tag="Vp")

        def q_proj(blk):
            acc = ppp.tile([128, QBLK], F32, tag="pp", name="qp")
            sl = slice(blk * QBLK, (blk + 1) * QBLK)
            for c in range(NCH):
                nc.tensor.matmul(
                    acc[:, :], wq_sb[:, c * 128:(c + 1) * 128],
                    qt_slice(c, blk),
                    start=(c == 0), stop=(c == NCH - 1))
            nc.vector.tensor_add(QT2[:, sl], acc[:, :], bq_sb[:, :])

        def k_moving(c, kt2b):      # 256-col raw-K slice for projection
            if kt2b == 0:
                return kqa[:, c, :]
            if kt2b == 1:
                return kqb[:, c, :]
            src = (kq1, kq2, kq3)[(kt2b - 2) // 2]
            off = ((kt2b - 2) % 2) * 256
            return src[:, c, off:off + 256]

        def k_proj(kt2b):           # 256-col K projection sub-block
            acc = ppp.tile([128, QBLK], F32, tag="pp", name="kp")
            sl = slice(kt2b * 256, (kt2b + 1) * 256)
            for c in range(NCH):
                nc.tensor.matmul(
                    acc[:, 0:256], wk_sb[:, c * 128:(c + 1) * 128],
                    k_moving(c, kt2b),
                    start=(c == 0), stop=(c == NCH - 1))
            nc.vector.tensor_copy(KT2[:, sl], acc[:, 0:256])

        def v_proj(t):
            acc = ppp.tile([128, QBLK], F32, tag="pp", name="vp")
            for c in range(NCH):
                nc.tensor.matmul(
                    acc[:, 0:E1], vt_slice(c, t),
                    wv_sb[:, c * E1:(c + 1) * E1],
                    start=(c == 0), stop=(c == NCH - 1))
            nc.vector.tensor_add(Vp[:, t, :], acc[:, 0:E1], bvb_sb[:, :])

        # --- attention over global kv-tile index T = blk*NKV + t.
        # Groups of GRP=2 tiles (one parity pair) share one psum slot +
        # one exp ACTIVATE; AVs lag one group behind.
        state = {"sc": None, "au": None, "al": None}
        pt_of = {}

        def scores(T):
            blk, t = divmod(T, NKV)
            p = T % GRP
            if p == 0:
                state["sc"] = scp.tile([128, GRP * QBLK], F32, tag="sc",
                                       name="sc")
            sq = slice(blk * QBLK, (blk + 1) * QBLK)
            half = T % 2
            nc.tensor.matmul(
                state["sc"][:, p * QBLK:(p + 1) * QBLK],
                KT2[half * E:(half + 1) * E, t * 128:(t + 1) * 128],
                QT2[half * E:(half + 1) * E, sq],
                start=True, stop=True, tile_position=(half * E, 0))

        def exp_group(g):
            pt = ptp.tile([128, GRP * QBLK], BF16, tag="pt", name="pt")
            nc.scalar.activation(
                pt[:, :], state["sc"][:, :],
                mybir.ActivationFunctionType.Exp, scale=0.125)
            pt_of[g] = pt

        def av(T):
            blk, t = divmod(T, NKV)
            p = T % GRP
            if t == 0:
                state["au"] = accup.tile([E1, QBLK], F32, tag="au", name="au")
                state["al"] = acclp.tile([E1, QBLK], F32, tag="al", name="al")
            pt = pt_of[T // GRP]
            nc.tensor.matmul(
                state["au"][:, :], Vp[0:64, t, :],
                pt[0:64, p * QBLK:(p + 1) * QBLK],
                start=(t == 0), stop=(t == NKV - 1), tile_position=(0, 0))
            nc.tensor.matmul(
                state["al"][:, :], Vp[64:128, t, :],
                pt[64:128, p * QBLK:(p + 1) * QBLK],
                start=(t == 0), stop=(t == NKV - 1), tile_position=(64, 0))

        def epilogue(blk):
            sq = slice(blk * QBLK, (blk + 1) * QBLK)
            ob = obp.tile([E1, QBLK], F32, tag="ob", name="ob")
            nc.vector.tensor_copy(ob[:, :], state["au"][:, :])
            nc.vector.tensor_add(ob[:, :], ob[:, :], state["al"][:, :])
            nc.gpsimd.dma_start(out=out[:, sq], in_=ob[:, :])

        def sc_group(g):
            scores(2 * g)
            scores(2 * g + 1)
            exp_group(g)

        def av_group(g):
            for T in (2 * g, 2 * g + 1):
                ab, at = divmod(T, NKV)
                if at == 0 and ab > 0:
                    epilogue(ab - 1)
                av(T)

        # --- schedule. K/Q-critical chain first; V strictly after the
        # attention front so a late V DMA can't stall the in-order PE queue.
        k_proj(0)                       # KT2 cols 0-255 (tiles 0,1)
        q_proj(0)
        sc_group(0)
        k_proj(1)                       # tiles 2,3
        sc_group(1)
        v_proj(0); v_proj(1)
        av_group(0)
        k_proj(2); k_proj(3)            # tiles 4-7
        sc_group(2)
        v_proj(2); v_proj(3)
        av_group(1)
        k_proj(4); k_proj(5)            # tiles 8-11
        sc_group(3)
        v_proj(4); v_proj(5)
        av_group(2)
        k_proj(6); k_proj(7)            # tiles 12-15
        sc_group(4)
        v_proj(6); v_proj(7)
        av_group(3)
        sc_group(5)
        v_proj(8); v_proj(9)
        av_group(4)
        sc_group(6)
        v_proj(10); v_proj(11)
        av_group(5)
        q_proj(1)
        sc_group(7)
        v_proj(12); v_proj(13)
        av_group(6)
        sc_group(8)
        v_proj(14); v_proj(15)
        av_group(7)

        for g in range(9, NGRP):
            sc_group(g)
            ab, at = divmod(2 * (g - 1), NKV)
            if at == 0 and ab + 1 < NBLK:
                q_proj(ab + 1)
            av_group(g - 1)
        av_group(NGRP - 1)
        epilogue(NBLK - 1)


_CACHED_NC = None


def _get_nc():
    global _CACHED_NC
    if _CACHED_NC is None:
        _CACHED_NC = _build_bass()
    return _CACHED_NC


def _swizzle_w(w: np.ndarray) -> np.ndarray:
    """[512, width] -> [128, NCH*width] with chunk-major free dim."""
    width = w.shape[1]
    return np.ascontiguousarray(
        w.reshape(NCH, 128, width).transpose(1, 0, 2).reshape(128, NCH * width)
    ).astype(ml_dtypes.bfloat16)


def _make_in_maps(q, k, v, Wq, bq, Wk, bk, Wv, bv):
    del bk  # constant along the kv axis -> softmax-invariant, dropped
    bf = ml_dtypes.bfloat16
    wq_d = np.concatenate([np.asarray(Wq, np.float32)] * 2, axis=1)
    wk_d = np.concatenate([np.asarray(Wk, np.float32)] * 2, axis=1)
    wq_s = _swizzle_w(wq_d)
    wk_s = _swizzle_w(wk_d)
    wv_aug = np.concatenate(
        [np.asarray(Wv, np.float32), np.zeros((D, 1), np.float32)], axis=1
    )
    wv_s = _swizzle_w(wv_aug)
    bq_col = np.asarray(bq, np.float32).reshape(E, 1)
    bq_a = np.ascontiguousarray(np.broadcast_to(
        np.concatenate([bq_col, bq_col], axis=0), (2 * E, QBLK))).astype(bf)
    bvb_row = np.concatenate([np.asarray(bv, np.float32), [1.0]]).astype(np.float32)
    bvb_a = np.ascontiguousarray(np.broadcast_to(bvb_row, (128, E1)))

    in_maps = []
    for core in range(N_CORES):
        b, h = core // 2, core % 2
        kh = np.asarray(k[b, h * KS:(h + 1) * KS, :], np.float32)
        vh = np.asarray(v[b, h * KS:(h + 1) * KS, :], np.float32)
        in_maps.append({
            "qT": np.ascontiguousarray(np.asarray(q[b], np.float32).T).astype(bf),
            "kT": np.ascontiguousarray(kh.T).astype(bf),
            "vT": np.ascontiguousarray(vh.T).astype(bf),
            "wq": wq_s, "wk": wk_s, "wv": wv_s,
            "bqb": bq_a, "bvb": bvb_a,
        })
    return in_maps


def _unshard(results) -> np.ndarray:
    final = np.empty((B, S, E), np.float32)
    for b in range(B):
        o = (np.asarray(results[2 * b]["out"], np.float32)
             + np.asarray(results[2 * b + 1]["out"], np.float32))  # [65, S]
        final[b] = (o[:E] / o[E:E + 1]).T
    return final


def kernel(q, k, v, Wq, bq, Wk, bk, Wv, bv, _trace=False):
    nc = _get_nc()
    in_maps = _make_in_maps(q, k, v, Wq, bq, Wk, bk, Wv, bv)
    res = run_bass_kernel_spmd(nc, in_maps, core_ids=list(range(N_CORES)),
                               trace=_trace)
    outp = _unshard(res.results)
    if _trace:
        kernel.last_result = res
    return outp


# revision 14
# speedup vs baseline: 1.2274x; 1.2274x over previous
"""Trainium2 Bass kernel for batched single-head attention with projections.

Reference computation (per batch b):
    Q = q @ Wq + bq ; K = k @ Wk + bk ; V = v @ Wv + bv        (512 -> 64)
    out = softmax(Q K^T / 8) V                                  (S = 4096)

Sharding: 8 cores = 4 batches x 2 kv-sequence halves. Each core gets
its full q (transposed, bf16) plus half of k,v for its batch (transposed,
bf16). Cores emit unnormalized numerator + denominator; host combines.

Device-side layout (transposed space):
  Q.T [128, 4096] = (Wq|Wq).T @ qT (+bq)   rows 64..127 duplicate 0..63
  K.T [128, 2048] = (Wk|Wk).T @ kT         (bk dropped: softmax-invariant)
  V'  [2048, 65]  = vT.T @ Wv_aug + bias ; col 64 == 1.0 (denominator col)
  per kv-tile T (128 kv rows x 512 q): scores.T -> PSUM, exp -> bf16 SBUF,
  V'.T @ P.T accumulated into [65, 512] per q-block.

Perf structure (v4):
  - ScalarE is the floor engine (~8.5M exps/core at 1 elem/lane/cycle);
    it does exps only. scores PSUM = two 3-bank slots; one ACTIVATE per
    3-tile group (N=1536) ping-ponging the slots.
  - kv-tiles pair by parity into PE row halves (tile_position row tiling)
    so the two 64-contraction scores MMs run concurrently. AV keeps full
    128-contraction (splitting it doubles serialized LDWEIGHTS - v3 data).
  - PE warm-up matmuls on memset data flip the HAM clock gate to 2.4 GHz
    before the first input DMA lands.
  - DMA arrival order == need order. Early HBM demand oversubscribes the
    ~358 GB/s/core budget, so transfers are split fine (K in 256-col
    quarters-first, Q block-1 rides the scalar ring, V in 512-col
    quarters) and AVs lag scores by 2 groups to buy slack.
"""

import numpy as np
import ml_dtypes

import concourse.bass as bass
import concourse.tile as tile
from concourse import mybir
from concourse.bass_utils import run_bass_kernel_spmd

BF16 = mybir.dt.bfloat16
F32 = mybir.dt.float32

B, S, D, E = 4, 4096, 512, 64
H = S                 # q rows per core (full sequence)
KS = S // 2           # kv rows per core (half sequence)
E1 = E + 1            # V' width (ones column appended)
NCH = D // 128        # contraction chunks (4)
NKV = KS // 128       # kv tiles per core (16)
QBLK = 512            # q columns per block
NBLK = H // QBLK      # 8
NT = NBLK * NKV       # global tile count (128)
GRP = 3               # kv-tiles per exp group / psum slot
NGRP = (NT + GRP - 1) // GRP    # 43 (last group has 2 tiles)
LAG = 2               # groups by which AV trails scores/exp
N_CORES = 8


def _build_bass(split_waits: bool = True) -> bass.Bass:
    nc = bass.Bass()
    qT = nc.declare_dram_parameter("qT", [D, H], BF16, isOutput=False)
    kT = nc.declare_dram_parameter("kT", [D, KS], BF16, isOutput=False)
    vT = nc.declare_dram_parameter("vT", [D, KS], BF16, isOutput=False)
    # weights pre-swizzled on host to [128, chunk*width] (partition-major);
    # wq/wk have their 64 columns duplicated -> 128-wide stationary
    wq = nc.declare_dram_parameter("wq", [128, NCH * 2 * E], BF16, isOutput=False)
    wk = nc.declare_dram_parameter("wk", [128, NCH * 2 * E], BF16, isOutput=False)
    wv = nc.declare_dram_parameter("wv", [128, NCH * E1], BF16, isOutput=False)
    bq = nc.declare_dram_parameter("bqb", [128, QBLK], BF16, isOutput=False)
    bvb = nc.declare_dram_parameter("bvb", [128, E1], F32, isOutput=False)
    out = nc.declare_dram_parameter("out", [E1, H], F32, isOutput=True)

    with tile.TileContext(nc) as tc:
        _body(nc, tc, qT, kT, vT, wq, wk, wv, bq, bvb, out)
    if split_waits:
        _split_multi_waits(nc)
    return nc


_NO_SPLIT_OPCODES = {"Drain", "EventSemaphore", "NoOp", "Call", "ISA",
                     "UnconditionalBranch"}


def _split_multi_waits(nc):
    """walrus (this toolchain) encodes at most ONE sem wait per TPB
    instruction (single NEURON_ISA_TPB_EVENTS slot) and refuses to compile
    instructions carrying more. Tile emits multi-wait sync_info freely, so
    split: keep the first wait on the instruction, hoist the rest onto
    standalone EventSemaphore waits just before it on the same engine."""
    n = 0
    for blk in nc.m.functions[0].blocks:
        new_insts = []
        for inst in blk.instructions:
            si = inst.sync_info
            if (si is not None and si.on_wait and len(si.on_wait) > 1
                    and inst.concise_opcode not in _NO_SPLIT_OPCODES):
                waits = list(si.on_wait)
                for w in waits[:-1]:
                    n += 1
                    es = mybir.InstEventSemaphore(
                        name=f"WSPLIT-{n}", ins=[], outs=[])
                    es.engine = inst.engine
                    es.sync_info = mybir.SyncInfo(on_wait=[w], on_update=[])
                    new_insts.append(es)
                inst.sync_info = mybir.SyncInfo(
                    on_wait=[waits[-1]], on_update=list(si.on_update))
            new_insts.append(inst)
        blk.instructions = new_insts
    return nc


def _body(nc, tc, qT, kT, vT, wq, wk, wv, bq, bvb, out):
    with (
        tc.tile_pool(name="consts", bufs=1) as cst,
        tc.tile_pool(name="raw", bufs=1) as raw,
        tc.tile_pool(name="proj", bufs=1) as proj,
        tc.tile_pool(name="pt", bufs=6) as ptp,
        tc.tile_pool(name="ob", bufs=2) as obp,
        tc.tile_pool(name="sc", bufs=2, space="PSUM") as scp,
        tc.tile_pool(name="acc", bufs=1, space="PSUM") as accp,
        tc.tile_pool(name="pp", bufs=1, space="PSUM") as ppp,
    ):
        # --- input DMAs, fanned out across rings; within each ring the
        # row drains FIFO, so issue order == arrival order == need order.
        def load3d(eng, name, src, c0, c1):
            w = c1 - c0
            t = raw.tile([128, NCH, w], BF16, tag=name, name=name)
            eng.dma_start(
                out=t,
                in_=src[:, c0:c1].rearrange("(c p) w -> p c w", p=128))
            return t

        # gpsimd (SWDGE): small weight tensors, then V in quarters
        wk_sb = cst.tile([128, NCH * 2 * E], BF16, tag="wk")
        nc.gpsimd.dma_start(out=wk_sb, in_=wk[:, :])
        wq_sb = cst.tile([128, NCH * 2 * E], BF16, tag="wq")
        nc.gpsimd.dma_start(out=wq_sb, in_=wq[:, :])
        bq_sb = cst.tile([128, QBLK], BF16, tag="bq")
        nc.gpsimd.dma_start(out=bq_sb, in_=bq[:, :])
        wv_sb = cst.tile([128, NCH * E1], BF16, tag="wv")
        nc.gpsimd.dma_start(out=wv_sb, in_=wv[:, :])
        bvb_sb = cst.tile([128, E1], F32, tag="bvb")
        nc.gpsimd.dma_start(out=bvb_sb, in_=bvb[:, :])
        vqs = [load3d(nc.gpsimd, f"vq{i}", vT, i * 512, (i + 1) * 512)
               for i in range(4)]

        # sync ring: K in need-order (first 256 kv cols alone so the first
        # scores group starts early), then the q tail
        kqa = load3d(nc.sync, "kqa", kT, 0, 256)
        kqb = load3d(nc.sync, "kqb", kT, 256, 512)
        kq1 = load3d(nc.sync, "kq1", kT, 512, 1024)
        kq2 = load3d(nc.sync, "kq2", kT, 1024, 1536)
        kq3 = load3d(nc.sync, "kq3", kT, 1536, 2048)
        qt_r2 = load3d(nc.sync, "qt_r2", qT, 1024, 2560)
        qt_r3 = load3d(nc.sync, "qt_r3", qT, 2560, H)

        # scalar ring: q blocks 0-1 only; ScalarE then does exps only
        qt_b0 = load3d(nc.scalar, "qt_b0", qT, 0, 512)
        qt_b1 = load3d(nc.scalar, "qt_b1", qT, 512, 1024)

        # PE warm-up: ~3.5us of matmuls on memset garbage flips the HAM
        # clock gate (1.2 -> 2.4 GHz) while the input DMAs are in flight.
        warm = cst.tile([128, 256], BF16, tag="warm")
        nc.vector.memset(warm, 0.0)
        wacc = ppp.tile([128, QBLK], F32, tag="pp", name="wacc")
        for i in range(12):
            nc.tensor.matmul(wacc[:, 0:256], warm[:, 0:128], warm[:, :],
                             start=(i == 0), stop=(i == 11))

        # preload the exp table set (first real exp otherwise eats the
        # ~2.7us ACT_TABLE_LOAD mid-pipeline); reads memset data
        scr = cst.tile([1, 8], F32, tag="scr")
        nc.scalar.activation(scr[:, :], warm[0:1, 0:8],
                             mybir.ActivationFunctionType.Exp)

        def qt_slice(c, blk):
            if blk == 0:
                return qt_b0[:, c, :]
            if blk == 1:
                return qt_b1[:, c, :]
            if blk < 5:
                return qt_r2[:, c, (blk - 2) * 512:(blk - 1) * 512]
            return qt_r3[:, c, (blk - 5) * 512:(blk - 4) * 512]

        def k_moving(c, kt2b):      # 256-col raw-K slice for projection
            if kt2b == 0:
                return kqa[:, c, :]
            if kt2b == 1:
                return kqb[:, c, :]
            src = (kq1, kq2, kq3)[(kt2b - 2) // 2]
            off = ((kt2b - 2) % 2) * 256
            return src[:, c, off:off + 256]

        def vt_slice(c, t):         # 128-col V tile
            return vqs[t // 4][:, c, (t % 4) * 128:(t % 4 + 1) * 128]

        # projected tensors; Q.T/K.T rows 64..127 duplicate rows 0..63 (via
        # column-duplicated weights) so scores matmuls can row-pack.
        QT2 = proj.tile([128, H], BF16, tag="QT2")
        KT2 = proj.tile([128, KS], BF16, tag="KT2")
        Vp = proj.tile([128, NKV, E1], BF16, tag="Vp")

        def q_proj(blk):
            acc = ppp.tile([128, QBLK], F32, tag="pp", name="qp")
            sl = slice(blk * QBLK, (blk + 1) * QBLK)
            for c in range(NCH):
                nc.tensor.matmul(
                    acc[:, :], wq_sb[:, c * 128:(c + 1) * 128],
                    qt_slice(c, blk),
                    start=(c == 0), stop=(c == NCH - 1))
            nc.vector.tensor_add(QT2[:, sl], acc[:, :], bq_sb[:, :])

        def k_proj(kt2b):           # 256-col K projection sub-block
            acc = ppp.tile([128, QBLK], F32, tag="pp", name="kp")
            sl = slice(kt2b * 256, (kt2b + 1) * 256)
            for c in range(NCH):
                nc.tensor.matmul(
                    acc[:, 0:256], wk_sb[:, c * 128:(c + 1) * 128],
                    k_moving(c, kt2b),
                    start=(c == 0), stop=(c == NCH - 1))
            nc.vector.tensor_copy(KT2[:, sl], acc[:, 0:256])

        def v_proj(t):
            acc = ppp.tile([128, QBLK], F32, tag="pp", name="vp")
            for c in range(NCH):
                nc.tensor.matmul(
                    acc[:, 0:E1], vt_slice(c, t),
                    wv_sb[:, c * E1:(c + 1) * E1],
                    start=(c == 0), stop=(c == NCH - 1))
            nc.vector.tensor_add(Vp[:, t, :], acc[:, 0:E1], bvb_sb[:, :])

        # --- attention over global kv-tile index T = blk*NKV + t.
        # Groups of GRP=3 tiles share one psum slot + one exp ACTIVATE.
        state = {"sc": None, "acc": None}
        pt_of = {}

        def scores(T):
            blk, t = divmod(T, NKV)
            p = T % GRP
            if p == 0:
                state["sc"] = scp.tile([128, GRP * QBLK], F32, tag="sc",
                                       name="sc")
            sq = slice(blk * QBLK, (blk + 1) * QBLK)
            half = T % 2
            nc.tensor.matmul(
                state["sc"][:, p * QBLK:(p + 1) * QBLK],
                KT2[half * E:(half + 1) * E, t * 128:(t + 1) * 128],
                QT2[half * E:(half + 1) * E, sq],
                start=True, stop=True, tile_position=(half * E, 0))

        def exp_group(g):
            lastT = min((g + 1) * GRP, NT) - 1
            n = (lastT % GRP) + 1
            pt = ptp.tile([128, GRP * QBLK], BF16, tag="pt", name="pt")
            nc.scalar.activation(
                pt[:, 0:n * QBLK], state["sc"][:, 0:n * QBLK],
                mybir.ActivationFunctionType.Exp, scale=0.125)
            pt_of[g] = pt

        def av(T):
            blk, t = divmod(T, NKV)
            p = T % GRP
            if t == 0:
                state["acc"] = accp.tile([E1, QBLK], F32, tag="acc",
                                         name="acc")
            nc.tensor.matmul(
                state["acc"][:, :], Vp[:, t, :],
                pt_of[T // GRP][:, p * QBLK:(p + 1) * QBLK],
                start=(t == 0), stop=(t == NKV - 1))

        def epilogue(blk):
            sq = slice(blk * QBLK, (blk + 1) * QBLK)
            ob = obp.tile([E1, QBLK], F32, tag="ob", name="ob")
            nc.vector.tensor_copy(ob[:, :], state["acc"][:, :])
            nc.gpsimd.dma_start(out=out[:, sq], in_=ob[:, :])

        def sc_group(g):
            for T in range(g * GRP, min((g + 1) * GRP, NT)):
                scores(T)
            exp_group(g)

        def av_group(g):
            for T in range(g * GRP, min((g + 1) * GRP, NT)):
                ab, at = divmod(T, NKV)
                if at == 0 and ab > 0:
                    epilogue(ab - 1)
                    if ab + 1 < NBLK:
                        q_proj(ab + 1)
                av(T)

        # --- schedule. K/Q-critical chain first; V strictly after the
        # attention front so a late V DMA can't stall the in-order PE
        # queue; AVs trail scores by LAG=2 groups.
        k_proj(0)                       # KT2 cols 0-255 (kv tiles 0,1)
        q_proj(0)
        scores(0); scores(1)
        k_proj(1)                       # tiles 2,3
        scores(2); exp_group(0)
        k_proj(2); k_proj(3)            # tiles 4-7
        sc_group(1)
        k_proj(4); k_proj(5)            # tiles 8-11
        sc_group(2)
        v_proj(0); v_proj(1); v_proj(2)
        av_group(0)
        k_proj(6); k_proj(7)            # tiles 12-15
        sc_group(3)
        v_proj(3); v_proj(4); v_proj(5)
        av_group(1)
        sc_group(4)
        v_proj(6); v_proj(7); v_proj(8)
        av_group(2)
        q_proj(1)
        sc_group(5)
        v_proj(9); v_proj(10); v_proj(11)
        av_group(3)
        sc_group(6)
        v_proj(12); v_proj(13); v_proj(14)
        av_group(4)
        sc_group(7)
        v_proj(15)
        av_group(5)

        for g in range(8, NGRP):
            sc_group(g)
            av_group(g - LAG)
        for g in range(NGRP - LAG, NGRP):
            av_group(g)
        epilogue(NBLK - 1)


_CACHED_NC = None


def _get_nc():
    global _CACHED_NC
    if _CACHED_NC is None:
        _CACHED_NC = _build_bass()
    return _CACHED_NC


def _swizzle_w(w: np.ndarray) -> np.ndarray:
    """[512, width] -> [128, NCH*width] with chunk-major free dim."""
    width = w.shape[1]
    return np.ascontiguousarray(
        w.reshape(NCH, 128, width).transpose(1, 0, 2).reshape(128, NCH * width)
    ).astype(ml_dtypes.bfloat16)


def _make_in_maps(q, k, v, Wq, bq, Wk, bk, Wv, bv):
    del bk  # constant along the kv axis -> softmax-invariant, dropped
    bf = ml_dtypes.bfloat16
    wq_d = np.concatenate([np.asarray(Wq, np.float32)] * 2, axis=1)
    wk_d = np.concatenate([np.asarray(Wk, np.float32)] * 2, axis=1)
    wq_s = _swizzle_w(wq_d)
    wk_s = _swizzle_w(wk_d)
    wv_aug = np.concatenate(
        [np.asarray(Wv, np.float32), np.zeros((D, 1), np.float32)], axis=1
    )
    wv_s = _swizzle_w(wv_aug)
    bq_col = np.asarray(bq, np.float32).reshape(E, 1)
    bq_a = np.ascontiguousarray(np.broadcast_to(
        np.concatenate([bq_col, bq_col], axis=0), (2 * E, QBLK))).astype(bf)
    bvb_row = np.concatenate([np.asarray(bv, np.float32), [1.0]]).astype(np.float32)
    bvb_a = np.ascontiguousarray(np.broadcast_to(bvb_row, (128, E1)))

    in_maps = []
    for core in range(N_CORES):
        b, h = core // 2, core % 2
        kh = np.asarray(k[b, h * KS:(h + 1) * KS, :], np.float32)
        vh = np.asarray(v[b, h * KS:(h + 1) * KS, :], np.float32)
        in_maps.append({
            "qT": np.ascontiguousarray(np.asarray(q[b], np.float32).T).astype(bf),
            "kT": np.ascontiguousarray(kh.T).astype(bf),
            "vT": np.ascontiguousarray(vh.T).astype(bf),
            "wq": wq_s, "wk": wk_s, "wv": wv_s,
            "bqb": bq_a, "bvb": bvb_a,
        })
    return in_maps


def _unshard(results) -> np.ndarray:
    final = np.empty((B, S, E), np.float32)
    for b in range(B):
        o = (np.asarray(results[2 * b]["out"], np.float32)
             + np.asarray(results[2 * b + 1]["out"], np.float32))  # [65, S]
        final[b] = (o[:E] / o[E:E + 1]).T
    return final


def kernel(q, k, v, Wq, bq, Wk, bk, Wv, bv, _trace=False):
    nc = _get_nc()
    in_maps = _make_in_maps(q, k, v, Wq, bq, Wk, bk, Wv, bv)
    res = run_bass_kernel_spmd(nc, in_maps, core_ids=list(range(N_CORES)),
                               trace=_trace)
    outp = _unshard(res.results)
    if _trace:
        kernel.last_result = res
    return outp
